# revision 1
# baseline (speedup 1.0000x reference)
"""HGCN decoder on 8 trn2 NeuronCores.

Strategy: nodes are sorted by in-degree, grouped into 128-node tiles, and the
tiles are dealt round-robin across the 8 cores (graph-parallel by destination
node).  Each core:
  - runs the node-wise hyperbolic math (HypLinear / exp / log maps) on its
    4096 nodes, tile by tile, with the per-node scalar chains batched into
    [128, 32] arrays,
  - publishes its tangent-space table shard, AllGathers the full [32768, 64]
    table to DRAM,
  - aggregates messages with `dma_gather` (padded per-tile CSR: tile t gathers
    [128, K_t, 64] source rows in one indirect DMA) followed by a weighted
    strided reduce on the vector engine,
  - finishes with the euclidean readout matmul.
All graph preprocessing (permutation, padded neighbor tables, weight folding
of edge/node masks) happens host-side in numpy; the device only sees dense
tables.
"""

import numpy as np

N = 32768
E = 1015808
D = 64
C = 8          # cores
NL = N // C    # 4096 nodes per core
P = 128        # partitions / tile
T = NL // P    # 32 tiles per core
MAXN = 1.0 - 4e-3   # PROJ_EPS boundary for c=1
EPS = 1e-15
ART_CLIP = 1.0 - 1e-5
MAX_TANH = 15.0


def _build_tables(rows, cols, edge_mask, node_mask):
    """Permute nodes by degree, deal tiles round-robin to cores, and build the
    per-core padded gather tables (int16 indices wrapped the way
    InstDMAGatherAnt wants them) plus matching weight tables."""
    deg = np.bincount(rows, minlength=N)
    order = np.argsort(-deg, kind="stable")
    # global tile j -> core j%C, slot j//C ; permuted position of its p-th node
    perm = np.empty(N, dtype=np.int64)
    j = np.arange(N) // P                     # global tile of sorted rank r
    c = j % C
    t = j // C
    p = np.arange(N) % P
    perm[c * NL + t * P + p] = order          # perm[g] = original node id
    pos = np.empty(N, dtype=np.int64)
    pos[perm] = np.arange(N)

    # gather-table row id for permuted position g=(c,t,p):
    #   AllGather concatenates per-core [P, T*D] blocks, so
    #   row_id = c*NL + p*T + t
    gg = np.arange(N)
    gc, gr = gg // NL, gg % NL
    gt, gp_ = gr // P, gr % P
    rowid = gc * NL + gp_ * T + gt            # [g] -> table row
    dstpos = pos[rows]
    eorder = np.argsort(dstpos, kind="stable")
    src_sorted = rowid[pos[cols[eorder]]]     # gather table rows, 0..N-1
    w_sorted = edge_mask[eorder, 0].astype(np.float64)
    cnts = np.bincount(dstpos, minlength=N)
    offs = np.zeros(N + 1, dtype=np.int64)
    np.cumsum(cnts, out=offs[1:])

    # per-slot K: max count over the 8 cores' tiles in that slot
    cnts_g = cnts.reshape(C, T, P)
    Ks = np.maximum(cnts_g.max(axis=(0, 2)), 1).astype(np.int64)   # [T]

    IDXC = int(8 * Ks.sum())
    WTC = int(Ks.sum())
    idx_dev = np.zeros((C, P, IDXC), np.int16)
    wt_dev = np.zeros((C, P, WTC), np.float32)
    nm = node_mask[:, 0].astype(np.float64)
    ioff = woff = 0
    ar = None
    for t in range(T):
        K = int(Ks[t])
        if ar is None or ar.shape[1] != K:
            ar = np.arange(K)[None, :]
        for cc in range(C):
            base = cc * NL + t * P
            cn = cnts[base:base + P]
            take = offs[base:base + P][:, None] + ar          # [P, K]
            valid = ar < cn[:, None]
            take_c = np.minimum(take, E - 1)
            nb = np.where(valid, src_sorted[take_c], 0)
            wl = np.where(valid, w_sorted[take_c], 0.0)
            wl = wl * nm[perm[base:base + P]][:, None]
            il = nb.T.reshape(-1)                             # i = g*128+p
            ch = il.reshape(8 * K, 16).T                      # [16, 8K]
            idx_dev[cc, :, ioff:ioff + 8 * K] = np.tile(ch, (8, 1)).astype(np.int16)
            wt_dev[cc, :, woff:woff + K] = wl.astype(np.float32)
        ioff += 8 * K
        woff += K
    # pad counts per (core, slot, partition) for the pad-subtract path
    pc_dev = np.zeros((C, 1, T * P), np.float32)
    for t in range(T):
        K = int(Ks[t])
        for cc in range(C):
            base = cc * NL + t * P
            pc_dev[cc, 0, t * P:(t + 1) * P] = K - cnts[base:base + P]
    allones = bool(np.all(edge_mask == 1.0) and np.all(node_mask == 1.0))
    return perm, Ks, idx_dev, wt_dev, IDXC, WTC, pc_dev, allones


def _build_program(Ks, IDXC, WTC, use_wt=True, sim=False):
    import os
    import concourse.bacc as bacc
    import concourse.bass as bass
    import concourse.mybir as mybir
    import concourse.tile as tile
    from concourse import library_config
    from concourse.masks import make_identity

    f32 = mybir.dt.float32
    i16 = mybir.dt.int16
    AF = mybir.ActivationFunctionType
    OP = mybir.AluOpType
    X = mybir.AxisListType.X

    nc = bacc.Bacc("TRN2", target_bir_lowering=False, debug=False,
                   num_devices=1 if sim else C)
    ablate = set(os.environ.get("KABLATE", "").split(",")) if sim else set()

    h_in = nc.dram_tensor("h_in", [P, T * D], f32, kind="ExternalInput")
    idx_in = nc.dram_tensor("idx_in", [P, IDXC], i16, kind="ExternalInput")
    wt_in = nc.dram_tensor("wt_in", [P, WTC], f32, kind="ExternalInput")
    w0t_in = nc.dram_tensor("w0t_in", [D, D], f32, kind="ExternalInput")
    w1t_in = nc.dram_tensor("w1t_in", [D, D], f32, kind="ExternalInput")
    wot_in = nc.dram_tensor("wot_in", [D, 16], f32, kind="ExternalInput")
    pc_in = nc.dram_tensor("pc_in", [1, T * P], f32, kind="ExternalInput")
    out_dram = nc.dram_tensor("out", [P, T * 16], f32, kind="ExternalOutput")
    xt_shard = nc.dram_tensor("xt_shard", [P, T * D], f32)
    xt_table = nc.dram_tensor("xt_table", [N, D], f32, addr_space="Shared")
    groups = [list(range(C))]

    with tile.TileContext(nc) as tc:
        nc.gpsimd.load_library(library_config.mlp)
        import contextlib
        ctx = contextlib.ExitStack()
        with ctx:
            const = ctx.enter_context(tc.tile_pool(name="const", bufs=1))
            sqp = ctx.enter_context(tc.tile_pool(name="sq", bufs=3))
            xtp = ctx.enter_context(tc.tile_pool(name="xtp", bufs=3))
            gp = ctx.enter_context(tc.tile_pool(name="gp", bufs=3))
            scp = ctx.enter_context(tc.tile_pool(name="scp", bufs=2))
            psp = ctx.enter_context(tc.tile_pool(name="psp", bufs=2, space="PSUM"))
            psmv = ctx.enter_context(tc.tile_pool(name="psmv", bufs=2, space="PSUM"))

            ident = const.tile([P, P], f32)
            make_identity(nc, ident[:])
            idx_sb = const.tile([P, IDXC], i16)
            nc.sync.dma_start(out=idx_sb[:], in_=idx_in[:])
            wt_sb = const.tile([P, WTC], f32)
            nc.sync.dma_start(out=wt_sb[:], in_=wt_in[:])
            w0t_sb = const.tile([D, D], f32)
            nc.sync.dma_start(out=w0t_sb[:], in_=w0t_in[:])
            w1t_sb = const.tile([D, D], f32)
            nc.sync.dma_start(out=w1t_sb[:], in_=w1t_in[:])
            wot_sb = const.tile([D, 16], f32)
            nc.sync.dma_start(out=wot_sb[:], in_=wot_in[:])
            pc_sb = const.tile([1, T * P], f32)
            nc.sync.dma_start(out=pc_sb[:], in_=pc_in[:])

            x_sb = const.tile([P, T * D], f32)      # node state (manifold)
            mv_sb = const.tile([P, T * D], f32)     # W@x then xt (tangent msgs)
            agg_sb = const.tile([P, T * D], f32)    # aggregated tangent
            u_sb = const.tile([P, T * D], f32)      # relu'd tangent
            out_sb = const.tile([P, T * 16], f32)

            nc.sync.dma_start(out=x_sb[:], in_=h_in[:])

            def ts(t, w=D):
                return slice(t * w, (t + 1) * w)

            def artanh(dst, src):
                """dst = 0.5*ln((1+c)/(1-c)), c = min(src, ART_CLIP); src>=0."""
                cth = scp.tile([P, T], f32, tag="art_c")
                nc.vector.tensor_scalar_min(cth[:], src[:], ART_CLIP)
                pt = scp.tile([P, T], f32, tag="art_p")
                nc.scalar.activation(pt[:], cth[:], AF.Copy, bias=1.0)
                mt = scp.tile([P, T], f32, tag="art_m")
                nc.scalar.activation(mt[:], cth[:], AF.Copy, scale=-1.0, bias=1.0)
                rm = scp.tile([P, T], f32, tag="art_rm")
                nc.vector.reciprocal(rm[:], mt[:])
                nc.vector.tensor_tensor(pt[:], pt[:], rm[:], op=OP.mult)
                nc.scalar.activation(pt[:], pt[:], AF.Ln)
                nc.vector.tensor_scalar_mul(dst[:], pt[:], 0.5)

            def norm_from_sq(dst, src):
                nc.scalar.activation(dst[:], src[:], AF.Sqrt)
                nc.vector.tensor_scalar_max(dst[:], dst[:], EPS)

            def exp_proj_scale(dst, nrm):
                """dst = min(tanh(min(nrm,15)), MAXN) / nrm"""
                a = scp.tile([P, T], f32, tag="eps_a")
                nc.vector.tensor_scalar_min(a[:], nrm[:], MAX_TANH)
                nc.scalar.activation(a[:], a[:], AF.Tanh)
                nc.vector.tensor_scalar_min(a[:], a[:], MAXN)
                r = scp.tile([P, T], f32, tag="eps_r")
                nc.vector.reciprocal(r[:], nrm[:])
                nc.vector.tensor_tensor(dst[:], a[:], r[:], op=OP.mult)

            # ---- x0 = proj(expmap0(h)) --------------------------------------
            nh2 = scp.tile([P, T], f32, tag="nh2")
            for t in range(T):
                sq = sqp.tile([P, D], f32, tag="sq")
                nc.scalar.activation(sq[:], x_sb[:, ts(t)], AF.Square,
                                     accum_out=nh2[:, t:t + 1])
            nh = scp.tile([P, T], f32, tag="nh")
            norm_from_sq(nh, nh2)
            s0 = scp.tile([P, T], f32, tag="s0")
            exp_proj_scale(s0, nh)
            for t in range(T):
                nc.vector.tensor_scalar_mul(x_sb[:, ts(t)], x_sb[:, ts(t)],
                                            s0[:, t:t + 1])

            for layer in range(2):
                w_l = w0t_sb if layer == 0 else w1t_sb
                # ---- HypLinear + logmap0 (analytic combined scale) ----------
                xn2 = scp.tile([P, T], f32, tag="xn2")
                mxn2 = scp.tile([P, T], f32, tag="mxn2")
                for t in range(T):
                    sq = sqp.tile([P, D], f32, tag="sq")
                    nc.scalar.activation(sq[:], x_sb[:, ts(t)], AF.Square,
                                         accum_out=xn2[:, t:t + 1])
                    xT_ps = psp.tile([D, P], f32, tag="xT")
                    nc.tensor.transpose(out=xT_ps[:], in_=x_sb[:, ts(t)],
                                        identity=ident[:])
                    xT = xtp.tile([D, P], f32, tag="xT_sb")
                    nc.vector.tensor_copy(xT[:], xT_ps[:])
                    mv_ps = psmv.tile([P, D], f32, tag="mv")
                    nc.tensor.matmul(out=mv_ps[:], lhsT=xT[:], rhs=w_l[:],
                                     start=True, stop=True)
                    sq2 = sqp.tile([P, D], f32, tag="sq")
                    nc.scalar.activation(sq2[:], mv_ps[:], AF.Square,
                                         accum_out=mxn2[:, t:t + 1])
                    nc.vector.tensor_copy(mv_sb[:, ts(t)], mv_ps[:])
                xn = scp.tile([P, T], f32, tag="xn")
                norm_from_sq(xn, xn2)
                mxn = scp.tile([P, T], f32, tag="mxn")
                norm_from_sq(mxn, mxn2)
                at = scp.tile([P, T], f32, tag="at")
                artanh(at, xn)
                rx = scp.tile([P, T], f32, tag="rx")
                nc.vector.reciprocal(rx[:], xn[:])
                nc.vector.tensor_tensor(at[:], at[:], rx[:], op=OP.mult)
                nc.vector.tensor_tensor(at[:], at[:], mxn[:], op=OP.mult)
                nc.vector.tensor_scalar_min(at[:], at[:], MAX_TANH)
                nc.scalar.activation(at[:], at[:], AF.Tanh)
                nc.vector.tensor_scalar_min(at[:], at[:], MAXN)
                smsg = scp.tile([P, T], f32, tag="smsg")
                artanh(smsg, at)
                rmx = scp.tile([P, T], f32, tag="rmx")
                nc.vector.reciprocal(rmx[:], mxn[:])
                nc.vector.tensor_tensor(smsg[:], smsg[:], rmx[:], op=OP.mult)
                for t in range(T):
                    nc.vector.tensor_scalar_mul(mv_sb[:, ts(t)], mv_sb[:, ts(t)],
                                                smsg[:, t:t + 1])
                # publish shard and AllGather the full tangent table
                nc.sync.dma_start(out=xt_shard[:], in_=mv_sb[:])
                nobar = "nobar" in ablate
                if sim:
                    if not nobar:
                        tc.strict_bb_all_engine_barrier()
                    nc.sync.dma_start(
                        out=xt_table[0:NL, :].rearrange("(p x) d -> p x d", p=P),
                        in_=xt_shard[:].rearrange("p (x d) -> p x d", d=D))
                    if not nobar:
                        tc.strict_bb_all_engine_barrier()
                else:
                    if not nobar:
                        tc.strict_bb_all_engine_barrier()
                    nc.gpsimd.collective_compute(
                        "AllGather", mybir.AluOpType.bypass, replica_groups=groups,
                        ins=[xt_shard[:, :]], outs=[xt_table[:, :]])
                    if not nobar:
                        tc.strict_bb_all_engine_barrier()

                # ---- aggregation: gather + weighted reduce ------------------
                row0_sb = scp.tile([1, D], f32, tag="row0")
                if not use_wt:
                    nc.sync.dma_start(out=row0_sb[:], in_=xt_table[0:1, :])
                na2 = scp.tile([P, T], f32, tag="na2")
                ioff = woff = 0
                for t in range(T):
                    K = int(Ks[t])
                    g = gp.tile([P, K * D], f32, tag="G")
                    g3 = g[:].rearrange("p (k d) -> p k d", d=D)
                    if "gather" not in ablate:
                        nc.gpsimd.dma_gather(
                            g3, xt_table[:, :], idx_sb[:, ioff:ioff + 8 * K],
                            num_idxs=P * K, num_idxs_reg=P * K, elem_size=D,
                            single_packet=False)
                    if use_wt and "wtmul" not in ablate:
                        wt_ap = wt_sb[:, woff:woff + K]
                        wv = bass.AP(wt_ap.tensor, wt_ap.offset,
                                     list(wt_ap.ap) + [[0, D]])
                        nc.vector.tensor_tensor(g3, g3, wv, op=OP.mult)
                    if "reduce" not in ablate:
                        nc.vector.tensor_reduce(
                            agg_sb[:, ts(t)],
                            g[:].rearrange("p (k d) -> p d k", d=D),
                            axis=X, op=OP.add)
                    if not use_wt:
                        corr_ps = psmv.tile([P, D], f32, tag="mv")
                        nc.tensor.matmul(
                            out=corr_ps[:], lhsT=pc_sb[0:1, t * P:(t + 1) * P],
                            rhs=row0_sb[0:1, :], start=True, stop=True)
                        nc.vector.tensor_tensor(agg_sb[:, ts(t)],
                                                agg_sb[:, ts(t)], corr_ps[:],
                                                op=OP.subtract)
                    sq = sqp.tile([P, D], f32, tag="sq")
                    nc.scalar.activation(sq[:], agg_sb[:, ts(t)], AF.Square,
                                         accum_out=na2[:, t:t + 1])
                    ioff += 8 * K
                    woff += K
                # s2 = artanh(min(tanh(min(na,15)),MAXN)) / na
                na = scp.tile([P, T], f32, tag="na")
                norm_from_sq(na, na2)
                a2 = scp.tile([P, T], f32, tag="a2")
                nc.vector.tensor_scalar_min(a2[:], na[:], MAX_TANH)
                nc.scalar.activation(a2[:], a2[:], AF.Tanh)
                nc.vector.tensor_scalar_min(a2[:], a2[:], MAXN)
                s2 = scp.tile([P, T], f32, tag="s2")
                artanh(s2, a2)
                rna = scp.tile([P, T], f32, tag="rna")
                nc.vector.reciprocal(rna[:], na[:])
                nc.vector.tensor_tensor(s2[:], s2[:], rna[:], op=OP.mult)
                # u = relu(agg * s2); nu2 accum
                nu2 = scp.tile([P, T], f32, tag="nu2")
                for t in range(T):
                    nc.scalar.activation(u_sb[:, ts(t)], agg_sb[:, ts(t)],
                                         AF.Relu, scale=s2[:, t:t + 1])
                    sq = sqp.tile([P, D], f32, tag="sq")
                    nc.scalar.activation(sq[:], u_sb[:, ts(t)], AF.Square,
                                         accum_out=nu2[:, t:t + 1])
                nu = scp.tile([P, T], f32, tag="nu")
                norm_from_sq(nu, nu2)
                s3 = scp.tile([P, T], f32, tag="s3")
                exp_proj_scale(s3, nu)
                for t in range(T):
                    nc.vector.tensor_scalar_mul(x_sb[:, ts(t)], u_sb[:, ts(t)],
                                                s3[:, t:t + 1])

            # ---- readout: out = x @ W_out.T (b_out == 0) --------------------
            for t in range(T):
                xT_ps = psp.tile([D, P], f32, tag="xT")
                nc.tensor.transpose(out=xT_ps[:], in_=x_sb[:, ts(t)],
                                    identity=ident[:])
                xT = xtp.tile([D, P], f32, tag="xT_sb")
                nc.vector.tensor_copy(xT[:], xT_ps[:])
                o_ps = psmv.tile([P, 16], f32, tag="mv")
                nc.tensor.matmul(out=o_ps[:], lhsT=xT[:], rhs=wot_sb[:],
                                 start=True, stop=True)
                nc.vector.tensor_copy(out_sb[:, ts(t, 16)], o_ps[:])
            nc.sync.dma_start(out=out_dram[:], in_=out_sb[:])
    nc.compile()
    return nc


def kernel(h, distances, rows, cols, node_mask, edge_mask,
           W0, b0, W1, b1, W_out, b_out, _trace=False):
    from concourse.bass_utils import run_bass_kernel_spmd

    h = np.asarray(h, dtype=np.float32)
    rows = np.asarray(rows).astype(np.int64)
    cols = np.asarray(cols).astype(np.int64)
    node_mask = np.asarray(node_mask, dtype=np.float32)
    edge_mask = np.asarray(edge_mask, dtype=np.float32)
    assert not np.any(np.asarray(b0)) and not np.any(np.asarray(b1)) and \
        not np.any(np.asarray(b_out)), "nonzero biases unsupported"

    perm, Ks, idx_dev, wt_dev, IDXC, WTC, pc_dev, allones = _build_tables(
        rows, cols, edge_mask, node_mask)

    hp = h[perm].reshape(C, T, P, D).transpose(0, 2, 1, 3).reshape(C, P, T * D)
    w0t = np.ascontiguousarray(np.asarray(W0, np.float32).T)
    w1t = np.ascontiguousarray(np.asarray(W1, np.float32).T)
    wot = np.ascontiguousarray(np.asarray(W_out, np.float32).T)

    nc = _build_program(Ks, IDXC, WTC, use_wt=not allones)
    in_maps = [{
        "h_in": np.ascontiguousarray(hp[c]),
        "idx_in": idx_dev[c],
        "wt_in": wt_dev[c],
        "w0t_in": w0t, "w1t_in": w1t, "wot_in": wot,
        "pc_in": pc_dev[c],
    } for c in range(C)]
    res = run_bass_kernel_spmd(nc, in_maps, list(range(C)), trace=_trace)
    od = np.stack([res.results[c]["out"] for c in range(C)])
    od = od.reshape(C, P, T, 16).transpose(0, 2, 1, 3).reshape(N, 16)
    out = np.empty((N, 16), np.float32)
    out[perm] = od
    if _trace:
        return out, res
    return out



# revision 5
# speedup vs baseline: 1.3423x; 1.3423x over previous
"""HGCN decoder on 8 trn2 NeuronCores.

Strategy: nodes are sorted by in-degree, grouped into 128-node tiles, and the
tiles are dealt round-robin across the 8 cores (graph-parallel by destination
node).  Per layer each core publishes its tangent-space messages, AllGathers
the full [32768, 64] table to DRAM, then aggregates messages with `dma_gather`
(padded per-tile CSR) followed by a strided reduce on the vector engine.

This version software-pipelines all node-wise math INTO the gather phase
(tile groups of 4: gather -> reduce -> relu -> matmul -> scale overlap the
remaining gathers' DMA time), and collapses the hyperbolic scalar chains via
the identity artanh(min(tanh(min(y,15)), 1-eps)) == min(y, artanh(1-eps)),
so each layer's per-node math is just norms, min, reciprocal and multiplies
(no tanh/ln tables except one final tanh for the readout).  The layer
pipeline never materializes the on-manifold point: with u the tangent input,
r = relu-part and M = r @ W^T, the published tangent message is simply
min(nm * min(nu, A)/nu, A)/nm * M  (A = artanh(1-4e-3)).

All graph preprocessing (permutation, padded neighbor tables, weight folding
of edge/node masks, input transpose, first-layer input norms) happens
host-side in numpy; the device only sees dense tables.
"""

import numpy as np

N = 32768
E = 1015808
D = 64
C = 8          # cores
NL = N // C    # 4096 nodes per core
P = 128        # partitions / tile
T = NL // P    # 32 tiles per core
G = 4          # tile group for the pipeline / batched scalar chains
MAXN = 1.0 - 4e-3   # PROJ_EPS boundary for c=1
ART_MAXN = 3.1063030478757595   # artanh(1 - 4e-3)
MAX_TANH = 15.0
SQ_BIAS = 1e-12     # norm = sqrt(n2 + SQ_BIAS), replaces max(norm, EPS)


def _build_tables(rows, cols, edge_mask, node_mask):
    """Permute nodes by degree, deal tiles round-robin to cores, and build the
    per-core padded gather tables (int16 indices wrapped the way
    InstDMAGatherAnt wants them) plus matching weight tables."""
    deg = np.bincount(rows, minlength=N)
    order = np.argsort(-deg, kind="stable")
    # global tile j -> core j%C, slot j//C ; permuted position of its p-th node
    perm = np.empty(N, dtype=np.int64)
    j = np.arange(N) // P                     # global tile of sorted rank r
    c = j % C
    t = j // C
    p = np.arange(N) % P
    perm[c * NL + t * P + p] = order          # perm[g] = original node id
    pos = np.empty(N, dtype=np.int64)
    pos[perm] = np.arange(N)

    # gather-table row id for permuted position g=(c,t,p):
    #   AllGather concatenates per-core [P, T*D] blocks, so
    #   row_id = c*NL + p*T + t
    gg = np.arange(N)
    gc, gr = gg // NL, gg % NL
    gt, gp_ = gr // P, gr % P
    rowid = gc * NL + gp_ * T + gt            # [g] -> table row
    dstpos = pos[rows]
    eorder = np.argsort(dstpos, kind="stable")
    src_sorted = rowid[pos[cols[eorder]]]     # gather table rows, 0..N-1
    w_sorted = edge_mask[eorder, 0].astype(np.float64)
    cnts = np.bincount(dstpos, minlength=N)
    offs = np.zeros(N + 1, dtype=np.int64)
    np.cumsum(cnts, out=offs[1:])

    # per-slot K: max count over the 8 cores' tiles in that slot
    cnts_g = cnts.reshape(C, T, P)
    Ks = np.maximum(cnts_g.max(axis=(0, 2)), 1).astype(np.int64)   # [T]

    IDXC = int(8 * Ks.sum())
    WTC = int(Ks.sum())
    idx_dev = np.zeros((C, P, IDXC), np.int16)
    wt_dev = np.zeros((C, P, WTC), np.float32)
    nm = node_mask[:, 0].astype(np.float64)
    ioff = woff = 0
    ar = None
    for t in range(T):
        K = int(Ks[t])
        if ar is None or ar.shape[1] != K:
            ar = np.arange(K)[None, :]
        for cc in range(C):
            base = cc * NL + t * P
            cn = cnts[base:base + P]
            take = offs[base:base + P][:, None] + ar          # [P, K]
            valid = ar < cn[:, None]
            take_c = np.minimum(take, E - 1)
            nb = np.where(valid, src_sorted[take_c], 0)
            wl = np.where(valid, w_sorted[take_c], 0.0)
            wl = wl * nm[perm[base:base + P]][:, None]
            il = nb.T.reshape(-1)                             # i = g*128+p
            ch = il.reshape(8 * K, 16).T                      # [16, 8K]
            idx_dev[cc, :, ioff:ioff + 8 * K] = np.tile(ch, (8, 1)).astype(np.int16)
            wt_dev[cc, :, woff:woff + K] = wl.astype(np.float32)
        ioff += 8 * K
        woff += K
    # pad counts per (core, slot, partition) for the pad-subtract path
    pc_dev = np.zeros((C, 1, T * P), np.float32)
    for t in range(T):
        K = int(Ks[t])
        for cc in range(C):
            base = cc * NL + t * P
            pc_dev[cc, 0, t * P:(t + 1) * P] = K - cnts[base:base + P]
    allones = bool(np.all(edge_mask == 1.0) and np.all(node_mask == 1.0))
    return perm, Ks, idx_dev, wt_dev, IDXC, WTC, pc_dev, allones


def _build_program(Ks, IDXC, WTC, use_wt=True, sim=False):
    import concourse.bacc as bacc
    import concourse.bass as bass
    import concourse.mybir as mybir
    import concourse.tile as tile
    from concourse import library_config
    from concourse.masks import make_identity

    f32 = mybir.dt.float32
    i16 = mybir.dt.int16
    AF = mybir.ActivationFunctionType
    OP = mybir.AluOpType
    X = mybir.AxisListType.X
    A = ART_MAXN

    nc = bacc.Bacc("TRN2", target_bir_lowering=False, debug=False,
                   num_devices=1 if sim else C)

    ht_in = nc.dram_tensor("ht_in", [D, NL], f32, kind="ExternalInput")
    t1h_in = nc.dram_tensor("t1h_in", [P, T], f32, kind="ExternalInput")
    idx_in = nc.dram_tensor("idx_in", [P, IDXC], i16, kind="ExternalInput")
    wt_in = nc.dram_tensor("wt_in", [P, WTC], f32, kind="ExternalInput")
    w0t_in = nc.dram_tensor("w0t_in", [D, D], f32, kind="ExternalInput")
    w1t_in = nc.dram_tensor("w1t_in", [D, D], f32, kind="ExternalInput")
    wot_in = nc.dram_tensor("wot_in", [D, 16], f32, kind="ExternalInput")
    pc_in = nc.dram_tensor("pc_in", [1, T * P], f32, kind="ExternalInput")
    out_dram = nc.dram_tensor("out", [P, T * 16], f32, kind="ExternalOutput")
    xt_shard = nc.dram_tensor("xt_shard", [P, T * D], f32)
    xt_table = nc.dram_tensor("xt_table", [N, D], f32, addr_space="Shared")
    groups = [list(range(C))]

    ioffs = np.zeros(T, dtype=np.int64)
    woffs = np.zeros(T, dtype=np.int64)
    np.cumsum(8 * Ks[:-1], out=ioffs[1:])
    np.cumsum(Ks[:-1], out=woffs[1:])

    with tile.TileContext(nc) as tc:
        nc.gpsimd.load_library(library_config.mlp)
        import contextlib
        ctx = contextlib.ExitStack()
        with ctx:
            const = ctx.enter_context(tc.tile_pool(name="const", bufs=1))
            sqp = ctx.enter_context(tc.tile_pool(name="sq", bufs=2))
            gp = ctx.enter_context(tc.tile_pool(name="gp", bufs=4))
            aggp = ctx.enter_context(tc.tile_pool(name="agg", bufs=6))
            rp = ctx.enter_context(tc.tile_pool(name="rp", bufs=6))
            rtp = ctx.enter_context(tc.tile_pool(name="rtp", bufs=4))
            scp = ctx.enter_context(tc.tile_pool(name="scp", bufs=2))
            psT = ctx.enter_context(tc.tile_pool(name="psT", bufs=2, space="PSUM"))
            psmv = ctx.enter_context(tc.tile_pool(name="psmv", bufs=3, space="PSUM"))
            psc = ctx.enter_context(tc.tile_pool(name="psc", bufs=2, space="PSUM"))
            pso = ctx.enter_context(tc.tile_pool(name="pso", bufs=1, space="PSUM"))

            ident = const.tile([P, P], f32)
            make_identity(nc, ident[:])
            ht_sb = const.tile([D, NL], f32)
            nc.sync.dma_start(out=ht_sb[:], in_=ht_in[:])
            idx_sb = const.tile([P, IDXC], i16)
            nc.sync.dma_start(out=idx_sb[:], in_=idx_in[:])
            t1h_sb = const.tile([P, T], f32)
            nc.sync.dma_start(out=t1h_sb[:], in_=t1h_in[:])
            w0t_sb = const.tile([D, D], f32)
            nc.sync.dma_start(out=w0t_sb[:], in_=w0t_in[:])
            w1t_sb = const.tile([D, D], f32)
            nc.sync.dma_start(out=w1t_sb[:], in_=w1t_in[:])
            wot_sb = const.tile([D, 16], f32)
            nc.sync.dma_start(out=wot_sb[:], in_=wot_in[:])
            pc_sb = const.tile([1, T * P], f32)
            nc.sync.dma_start(out=pc_sb[:], in_=pc_in[:])
            if use_wt:
                wt_sb = const.tile([P, WTC], f32)
                nc.sync.dma_start(out=wt_sb[:], in_=wt_in[:])

            sqb = const.tile([P, 1], f32)            # bias inside sqrt
            nc.gpsimd.memset(sqb[:], SQ_BIAS)
            msg_sb = const.tile([P, T * D], f32)     # published tangent msgs
            out_sb = const.tile([P, T * 16], f32)
            # per-node scalar accumulators / chain temps, [P, T] column per tile
            na2 = const.tile([P, T], f32)
            nr2 = const.tile([P, T], f32)
            nm2 = const.tile([P, T], f32)
            nA = const.tile([P, T], f32)
            nR = const.tile([P, T], f32)
            nM = const.tile([P, T], f32)
            rX = const.tile([P, T], f32)
            s2 = const.tile([P, T], f32)
            nu = const.tile([P, T], f32)
            tmp = const.tile([P, T], f32)
            t1 = const.tile([P, T], f32)
            yv = const.tile([P, T], f32)
            sc = const.tile([P, T], f32)

            def ts(t, w=D):
                return slice(t * w, (t + 1) * w)

            def publish_table():
                nc.sync.dma_start(out=xt_shard[:], in_=msg_sb[:])
                tc.strict_bb_all_engine_barrier()
                if sim:
                    nc.sync.dma_start(
                        out=xt_table[0:NL, :].rearrange("(p x) d -> p x d", p=P),
                        in_=xt_shard[:].rearrange("p (x d) -> p x d", d=D))
                else:
                    nc.gpsimd.collective_compute(
                        "AllGather", mybir.AluOpType.bypass,
                        replica_groups=groups,
                        ins=[xt_shard[:, :]], outs=[xt_table[:, :]])
                tc.strict_bb_all_engine_barrier()

            # ---- head: msg0 = min(nm*t1h, A)/nm * (h @ W0^T) ----------------
            for g0 in range(0, T, G):
                tl = range(g0, g0 + G)
                sl = slice(g0, g0 + G)
                for t in tl:
                    mv = psmv.tile([P, D], f32, tag="mv")
                    nc.tensor.matmul(out=mv[:], lhsT=ht_sb[:, t * P:(t + 1) * P],
                                     rhs=w0t_sb[:], start=True, stop=True)
                    sq = sqp.tile([P, D], f32, tag="sq")
                    nc.scalar.activation(sq[:], mv[:], AF.Square,
                                         accum_out=nm2[:, t:t + 1])
                    nc.vector.tensor_copy(msg_sb[:, ts(t)], mv[:])
                nc.scalar.activation(nM[:, sl], nm2[:, sl], AF.Sqrt, bias=sqb[:])
                nc.vector.tensor_tensor(yv[:, sl], nM[:, sl], t1h_sb[:, sl],
                                        op=OP.mult)
                nc.vector.reciprocal(rX[:, sl], nM[:, sl])
                nc.vector.tensor_scalar_min(tmp[:, sl], yv[:, sl], A)
                nc.vector.tensor_tensor(sc[:, sl], tmp[:, sl], rX[:, sl],
                                        op=OP.mult)
                for t in tl:
                    nc.vector.tensor_scalar_mul(msg_sb[:, ts(t)],
                                                msg_sb[:, ts(t)],
                                                sc[:, t:t + 1])
            publish_table()

            # ---- two gather phases: layer-1 messages, then readout ----------
            for phase in range(2):
                produce = phase == 0
                w_rhs = w1t_sb if produce else wot_sb
                if not use_wt:
                    row0_sb = scp.tile([1, D], f32, tag="row0")
                    nc.sync.dma_start(out=row0_sb[:], in_=xt_table[0:1, :])
                for g0 in range(0, T, G):
                    tl = range(g0, g0 + G)
                    sl = slice(g0, g0 + G)
                    for t in tl:
                        K = int(Ks[t])
                        gbuf = gp.tile([P, K * D], f32, tag="G")
                        g3 = gbuf[:].rearrange("p (k d) -> p k d", d=D)
                        nc.gpsimd.dma_gather(
                            g3, xt_table[:, :],
                            idx_sb[:, int(ioffs[t]):int(ioffs[t]) + 8 * K],
                            num_idxs=P * K, num_idxs_reg=P * K, elem_size=D,
                            single_packet=False)
                        if use_wt:
                            wt_ap = wt_sb[:, int(woffs[t]):int(woffs[t]) + K]
                            wv = bass.AP(wt_ap.tensor, wt_ap.offset,
                                         list(wt_ap.ap) + [[0, D]])
                            nc.vector.tensor_tensor(g3, g3, wv, op=OP.mult)
                        agg = aggp.tile([P, D], f32, tag="agg")
                        nc.vector.tensor_reduce(
                            agg[:], gbuf[:].rearrange("p (k d) -> p d k", d=D),
                            axis=X, op=OP.add)
                        if not use_wt:
                            corr = psc.tile([P, D], f32, tag="corr")
                            nc.tensor.matmul(
                                out=corr[:], lhsT=pc_sb[0:1, t * P:(t + 1) * P],
                                rhs=row0_sb[0:1, :], start=True, stop=True)
                            nc.vector.tensor_tensor(agg[:], agg[:], corr[:],
                                                    op=OP.subtract)
                        sq = sqp.tile([P, D], f32, tag="sq")
                        nc.scalar.activation(sq[:], agg[:], AF.Square,
                                             accum_out=na2[:, t:t + 1])
                        r = rp.tile([P, D], f32, tag="r")
                        nc.scalar.activation(r[:], agg[:], AF.Relu)
                        sq2 = sqp.tile([P, D], f32, tag="sq")
                        nc.scalar.activation(sq2[:], r[:], AF.Square,
                                             accum_out=nr2[:, t:t + 1])
                        rT_ps = psT.tile([D, P], f32, tag="rT")
                        nc.tensor.transpose(out=rT_ps[:], in_=r[:],
                                            identity=ident[:])
                        rT = rtp.tile([D, P], f32, tag="rTs")
                        nc.vector.tensor_copy(rT[:], rT_ps[:])
                        if produce:
                            mv = psmv.tile([P, D], f32, tag="mv")
                            nc.tensor.matmul(out=mv[:], lhsT=rT[:], rhs=w_rhs[:],
                                             start=True, stop=True)
                            sq3 = sqp.tile([P, D], f32, tag="sq")
                            nc.scalar.activation(sq3[:], mv[:], AF.Square,
                                                 accum_out=nm2[:, t:t + 1])
                            nc.vector.tensor_copy(msg_sb[:, ts(t)], mv[:])
                        else:
                            o = pso.tile([P, 16], f32, tag="o")
                            nc.tensor.matmul(out=o[:], lhsT=rT[:], rhs=w_rhs[:],
                                             start=True, stop=True)
                            nc.vector.tensor_copy(out_sb[:, ts(t, 16)], o[:])
                    if produce:
                        # sc = min(nm * min(s2*nr, A)/nr, A)/nm ; s2=min(na,A)/na
                        nc.scalar.activation(nA[:, sl], na2[:, sl], AF.Sqrt,
                                             bias=sqb[:])
                        nc.scalar.activation(nR[:, sl], nr2[:, sl], AF.Sqrt,
                                             bias=sqb[:])
                        nc.scalar.activation(nM[:, sl], nm2[:, sl], AF.Sqrt,
                                             bias=sqb[:])
                        nc.vector.reciprocal(rX[:, sl], nA[:, sl])
                        nc.vector.tensor_scalar_min(tmp[:, sl], nA[:, sl], A)
                        nc.vector.tensor_tensor(s2[:, sl], tmp[:, sl],
                                                rX[:, sl], op=OP.mult)
                        nc.vector.tensor_tensor(nu[:, sl], s2[:, sl],
                                                nR[:, sl], op=OP.mult)
                        nc.vector.reciprocal(rX[:, sl], nR[:, sl])
                        nc.vector.tensor_scalar_min(tmp[:, sl], nu[:, sl], A)
                        nc.vector.tensor_tensor(t1[:, sl], tmp[:, sl],
                                                rX[:, sl], op=OP.mult)
                        nc.vector.tensor_tensor(yv[:, sl], nM[:, sl],
                                                t1[:, sl], op=OP.mult)
                        nc.vector.reciprocal(rX[:, sl], nM[:, sl])
                        nc.vector.tensor_scalar_min(tmp[:, sl], yv[:, sl], A)
                        nc.vector.tensor_tensor(sc[:, sl], tmp[:, sl],
                                                rX[:, sl], op=OP.mult)
                        for t in tl:
                            nc.vector.tensor_scalar_mul(msg_sb[:, ts(t)],
                                                        msg_sb[:, ts(t)],
                                                        sc[:, t:t + 1])
                if produce:
                    publish_table()

            # ---- readout scale: gam = min(tanh(min(s2*nr,15)), MAXN)/nr -----
            nc.scalar.activation(nA[:, :], na2[:, :], AF.Sqrt, bias=sqb[:])
            nc.scalar.activation(nR[:, :], nr2[:, :], AF.Sqrt, bias=sqb[:])
            nc.vector.reciprocal(rX[:, :], nA[:, :])
            nc.vector.tensor_scalar_min(tmp[:, :], nA[:, :], A)
            nc.vector.tensor_tensor(s2[:, :], tmp[:, :], rX[:, :], op=OP.mult)
            nc.vector.tensor_tensor(nu[:, :], s2[:, :], nR[:, :], op=OP.mult)
            nc.vector.tensor_scalar_min(tmp[:, :], nu[:, :], MAX_TANH)
            nc.scalar.activation(yv[:, :], tmp[:, :], AF.Tanh)
            nc.vector.tensor_scalar_min(yv[:, :], yv[:, :], MAXN)
            nc.vector.reciprocal(rX[:, :], nR[:, :])
            nc.vector.tensor_tensor(sc[:, :], yv[:, :], rX[:, :], op=OP.mult)
            for t in range(T):
                nc.vector.tensor_scalar_mul(out_sb[:, ts(t, 16)],
                                            out_sb[:, ts(t, 16)],
                                            sc[:, t:t + 1])
            nc.sync.dma_start(out=out_dram[:], in_=out_sb[:])
    nc.compile()
    return nc


def kernel(h, distances, rows, cols, node_mask, edge_mask,
           W0, b0, W1, b1, W_out, b_out, _trace=False):
    from concourse.bass_utils import run_bass_kernel_spmd

    h = np.asarray(h, dtype=np.float32)
    rows = np.asarray(rows).astype(np.int64)
    cols = np.asarray(cols).astype(np.int64)
    node_mask = np.asarray(node_mask, dtype=np.float32)
    edge_mask = np.asarray(edge_mask, dtype=np.float32)
    assert not np.any(np.asarray(b0)) and not np.any(np.asarray(b1)) and \
        not np.any(np.asarray(b_out)), "nonzero biases unsupported"

    perm, Ks, idx_dev, wt_dev, IDXC, WTC, pc_dev, allones = _build_tables(
        rows, cols, edge_mask, node_mask)

    hp = h[perm].reshape(C, T, P, D)
    ht = np.ascontiguousarray(hp.transpose(0, 3, 1, 2).reshape(C, D, NL))
    nh = np.sqrt((hp.astype(np.float64) ** 2).sum(-1))          # [C, T, P]
    t1h = (np.minimum(nh, ART_MAXN) / np.maximum(nh, 1e-15))
    t1h = np.ascontiguousarray(t1h.transpose(0, 2, 1)).astype(np.float32)
    w0t = np.ascontiguousarray(np.asarray(W0, np.float32).T)
    w1t = np.ascontiguousarray(np.asarray(W1, np.float32).T)
    wot = np.ascontiguousarray(np.asarray(W_out, np.float32).T)

    nc = _build_program(Ks, IDXC, WTC, use_wt=not allones)
    in_maps = [{
        "ht_in": ht[c],
        "t1h_in": t1h[c],
        "idx_in": idx_dev[c],
        "wt_in": wt_dev[c],
        "w0t_in": w0t, "w1t_in": w1t, "wot_in": wot,
        "pc_in": pc_dev[c],
    } for c in range(C)]
    res = run_bass_kernel_spmd(nc, in_maps, list(range(C)), trace=_trace)
    od = np.stack([res.results[c]["out"] for c in range(C)])
    od = od.reshape(C, P, T, 16).transpose(0, 2, 1, 3).reshape(N, 16)
    out = np.empty((N, 16), np.float32)
    out[perm] = od
    if _trace:
        return out, res
    return out
